# revision 5
# baseline (speedup 1.0000x reference)
"""DNA-Net GNN v2: single raw-history table + folded weights.

Key differences vs baseline kernel.py:
  - No V/K/Q per-layer tables. One H-table [NPAD, 256] bf16 holds the raw
    history (j-major col blocks, stored-PERM features). Per-layer Qt table
    [NPAD, 128] bf16 holds qtilde = (Wk^T Wq h_l + Wk^T bq)/sqrt(8) x2.
  - Scores from prod = hT * qT (same sel-matmul trick as baseline).
  - Aggregation one-hot is scaled by dis[row] (two-op tensor_scalar), so
    aggregation sums dis_row-weighted raw history; Wv/bv applied after the
    ReduceScatter on each core's shard (8x less work, no table writes).
  - Layer 0 also aggregates a ones column -> Z2[n] = sum_e dis[row_e],
    kept on-core (row 64 of zkeep) for the bv term of every layer.
  - Degrees/dis computed on host (pure edge_index preprocessing).
  - Blocks processed in core-interleaved order so ReduceScatter of the
    first 13/25 of every shard (ZtA) overlaps the edge-pass tail.
  - h is AllGathered row-major; Qt is built per-shard post-RS and
    AllGathered directly as the gather table.
"""

import math
import numpy as np

try:
    from ml_dtypes import bfloat16 as np_bf16
except ImportError:  # pragma: no cover
    np_bf16 = None

# ---------------------------------------------------------------- constants
N = 25000
E0 = 400000
HEADS = 8
DH = 8
HID = 64
F_IN = 256
N_CLASS = 32
N_LAYERS = 3
NCORES = 8

BLK = 128
NPAD = 25600
NB = NPAD // BLK            # 200
SHARD = NPAD // NCORES      # 3200
SBLK = SHARD // BLK         # 25
ABLK = 13                   # local blocks in ZtA (per core); B gets 12
BATCH = 4096

HW_TBL = 256                # H-table width (elements)
EW = {0: 128, 1: 128, 2: 256}   # edge-major gather elem size
TW = {1: 128, 2: 256}           # transposed gather elem size
ZW = {0: 65, 1: 64, 2: 64}      # Z row width (l0 carries Z2 column)

PERM = np.arange(64).reshape(8, 8).T.ravel()
ISQ = 1.0 / math.sqrt(DH)


# ---------------------------------------------------------------- host prep
def _block_diag(w):
    G = w.shape[0]
    out = np.zeros((64, 64), np.float32)
    for g in range(G):
        out[8 * g:8 * g + 8, 8 * g:8 * g + 8] = w[g]
    return out


def _perm_w(bd):
    return bd[PERM][:, PERM]


def _wrap16(idx, dtype=np.int16):
    E = idx.shape[0]
    assert E % 16 == 0
    w = idx.astype(dtype).reshape(E // 16, 16).T
    return np.tile(w, (8, 1))


def _pos_of_block(b):
    """node-block -> processing position (core-interleaved)."""
    return (b % SBLK) * NCORES + b // SBLK


def prep_edges(edge_index):
    """Edge streams ordered by interleaved block position.

    Returns rows (real), cols (real), sets [(pos, fs, ls, si)], EPAD, NG.
    """
    row = np.concatenate([edge_index[0], np.arange(N)]).astype(np.int64)
    col = np.concatenate([edge_index[1], np.arange(N)]).astype(np.int64)
    b = col // BLK
    pos = (b % SBLK) * NCORES + b // SBLK
    key = pos * BLK + (col % BLK)
    order = np.argsort(key, kind="stable")
    row, col, key = row[order], col[order], key[order]

    Tb = np.bincount(key // BLK, minlength=NB)       # edges per pos-block
    Ub = np.maximum((Tb + NCORES - 1) // NCORES, 1)
    Sb = np.concatenate([[0], np.cumsum(Ub)])
    total = int(Sb[-1])
    EPAD = ((total + BATCH - 1) // BATCH) * BATCH    # multiple of BATCH

    rows = np.zeros((NCORES, EPAD), np.int32)
    cols = np.full((NCORES, EPAD), -1, np.int32)
    keys = np.full((NCORES, EPAD), -1, np.int64)
    bstart = np.concatenate([[0], np.cumsum(Tb)])
    for p in range(NB):
        t = int(Tb[p])
        base, rem = divmod(t, NCORES)
        for c in range(NCORES):
            cnt = base + (1 if c < rem else 0)
            off = c * base + min(c, rem)
            s = int(Sb[p])
            rows[c, s:s + cnt] = row[bstart[p] + off: bstart[p] + off + cnt]
            cols[c, s:s + cnt] = col[bstart[p] + off: bstart[p] + off + cnt]
            keys[c, s:s + cnt] = key[bstart[p] + off: bstart[p] + off + cnt]

    sets = []
    NG = EPAD // BLK
    for p in range(NB):
        g0 = int(Sb[p]) // BLK
        g1 = (int(Sb[p + 1]) - 1) // BLK
        for g in range(g0, g1 + 1):
            sets.append((p, g, g == g0, g == g1))
    return dict(rows=rows, cols=cols, keys=keys, sets=sets, EPAD=EPAD, NG=NG)


def host_norm(edge_index):
    col = np.concatenate([edge_index[1], np.arange(N)]).astype(np.int64)
    deg = np.bincount(col, minlength=NPAD).astype(np.float32)
    degp = deg + (deg == 0)
    return (1.0 / np.sqrt(degp)).astype(np.float32)


def prep_weights(lin1_w, lin1_b, wq, bq, wk, bk, wv, bv, lin2_w, lin2_b):
    W = {}
    W["W1"] = lin1_w[:, PERM].astype(np.float32)
    W["b1"] = lin1_b[PERM].astype(np.float32)
    for l in range(1, N_LAYERS):
        Wq = _perm_w(_block_diag(wq[l]))
        Wk = _perm_w(_block_diag(wk[l]))
        W[f"Wt{l}"] = (Wk.T @ Wq * ISQ).astype(np.float32)
        W[f"bt{l}"] = (Wk.T @ (bq[l][PERM]) * ISQ).astype(np.float32)
    for l in range(N_LAYERS):
        W[f"Wv{l}"] = _perm_w(_block_diag(wv[l])).astype(np.float32)
        W[f"bv{l}"] = bv[l][PERM].astype(np.float32)
    W["W2"] = lin2_w[PERM].astype(np.float32)
    W["b2"] = lin2_b.astype(np.float32)
    return W


def _sel_matrices(L):
    sel_a = np.zeros((128, 16), np.float32)
    for p in range(128):
        lp, h = p // 64, p % 8
        sel_a[p, lp * 8 + h] = 1.0
    if L == 2:
        return sel_a, None
    sel_b = np.zeros((64, 8), np.float32)
    for p in range(64):
        sel_b[p, p % 8] = 1.0
    return sel_a, sel_b


# ================================================================ bass build
def _group_sets(sets, NG):
    per_g = [[] for _ in range(NG)]
    for si, (p, g, fs, ls) in enumerate(sets):
        per_g[g].append((p, fs, ls, si))
    return per_g


def build_nc(ep, hw=True):
    import contextlib
    import concourse.bass as bass
    import concourse.mybir as mybir
    import concourse.tile as tile

    dt = mybir.dt
    AF = mybir.ActivationFunctionType
    OP = mybir.AluOpType

    EPAD, NG, sets = ep["EPAD"], ep["NG"], ep["sets"]
    NSETS = len(sets)
    per_g = _group_sets(sets, NG)
    B = BATCH

    nc = bass.Bass(num_devices=NCORES)
    f32, bf16, i16 = dt.float32, dt.bfloat16, dt.int16

    # ---------------- I/O ----------------
    x_sh = nc.dram_tensor("x_sh", [SHARD, F_IN], f32, kind="ExternalInput")
    rows16 = nc.dram_tensor("rows16", [128, EPAD // 16], i16, kind="ExternalInput")
    cols16 = nc.dram_tensor("cols16", [128, EPAD // 16], i16, kind="ExternalInput")
    colp_in = nc.dram_tensor("colp", [128, NSETS], f32, kind="ExternalInput")
    dre_in = nc.dram_tensor("dre", [128, NG], f32, kind="ExternalInput")
    dissh_in = nc.dram_tensor("dissh", [128, SBLK], f32, kind="ExternalInput")
    iota_in = nc.dram_tensor("iota", [128, 128], bf16, kind="ExternalInput")
    identb_in = nc.dram_tensor("identb", [128, 128], bf16, kind="ExternalInput")
    sel2_in = nc.dram_tensor("sel2", [128, 16], bf16, kind="ExternalInput")
    sel3a_in = nc.dram_tensor("sel3a", [128, 16], bf16, kind="ExternalInput")
    sel3b_in = nc.dram_tensor("sel3b", [64, 8], bf16, kind="ExternalInput")
    W1_in = nc.dram_tensor("W1b", [F_IN, HID], bf16, kind="ExternalInput")
    b1r_in = nc.dram_tensor("b1r", [1, HID], f32, kind="ExternalInput")
    onesr_in = nc.dram_tensor("onesr", [1, 128], f32, kind="ExternalInput")
    Wtb_in = {l: nc.dram_tensor(f"Wtb{l}", [65, 64], bf16, kind="ExternalInput")
              for l in (1, 2)}
    Wvb_in = {l: nc.dram_tensor(f"Wvb{l}", [65, 64], bf16, kind="ExternalInput")
              for l in range(3)}
    W2b_in = nc.dram_tensor("W2b", [65, N_CLASS], bf16, kind="ExternalInput")
    lg_out = nc.dram_tensor("logits", [SHARD, N_CLASS], f32, kind="ExternalOutput")

    # ---------------- DRAM internals ----------------
    H = nc.dram_tensor("H", [NPAD, HW_TBL], bf16)
    hsh0 = nc.dram_tensor("hsh0", [SHARD, HID], bf16)
    hfull0 = nc.dram_tensor("hfull0", [NCORES, SHARD, HID], bf16,
                            addr_space="Shared")
    csh = {l: nc.dram_tensor(f"csh{l}", [SHARD, 256], bf16) for l in (1, 2)}
    Comb = {l: nc.dram_tensor(f"Comb{l}", [NCORES, SHARD, 256], bf16,
                              addr_space="Shared") for l in (1, 2)}
    qtab = {l: nc.dram_tensor(f"qtab{l}", [NPAD, 128], bf16) for l in (1, 2)}
    Zt = {l: nc.dram_tensor(f"Zt{l}", [NCORES * SBLK * 128, ZW[l]], bf16)
          for l in range(3)}
    Zrs = {l: nc.dram_tensor(f"Zrs{l}", [SBLK * 128, ZW[l]], bf16)
           for l in range(3)}
    RG = [list(range(NCORES))]

    with tile.TileContext(nc) as tc, contextlib.ExitStack() as ctx:
        if hw:
            po = nc.isa.get_enum("NEURON_ISA_TPB_PSEUDO_OPCODE")
            nc.gpsimd.isa(
                nc.isa.Opcode.NEURON_ISA_TPB_OPCODE_PSEUDO_INST,
                {"pseudo_opcode":
                 po.NEURON_ISA_TPB_PSEUDO_OPCODE_PSEUDO_LIBRARY_RELOAD_INDEX
                 .value,
                 "lib_index": 3},
                struct_name="NEURON_ISA_TPB_PSEUDO_LIBRARY_RELOAD_INDEX_STRUCT",
                verify=False)
        else:
            from concourse import library_config
            nc.gpsimd.load_library(library_config.mlp)
        _gregs = {}

        def greg(v):
            if v not in _gregs:
                _gregs[v] = nc.gpsimd.to_reg(v)
            return _gregs[v]

        cpool = ctx.enter_context(tc.tile_pool(name="const", bufs=1))
        iota = cpool.tile([128, 128], bf16, tag="iota")
        identb = cpool.tile([128, 128], bf16, tag="identb")
        sel2 = cpool.tile([128, 16], bf16, tag="sel2")
        sel3a = cpool.tile([128, 16], bf16, tag="sel3a")
        sel3b = cpool.tile([64, 8], bf16, tag="sel3b")
        W1t = cpool.tile([128, 2, HID], bf16, tag="W1t")
        b1tile = cpool.tile([128, HID], f32, tag="b1tile")
        Wtb = {l: cpool.tile([65, 64], bf16, tag=f"Wtb{l}", name=f"Wtb{l}")
               for l in (1, 2)}
        Wvb = {l: cpool.tile([65, 64], bf16, tag=f"Wvb{l}", name=f"Wvb{l}")
               for l in range(3)}
        W2b = cpool.tile([65, N_CLASS], bf16, tag="W2b")
        rows_sb = cpool.tile([128, EPAD // 16], i16, tag="rows_sb")
        cols_sb = cpool.tile([128, EPAD // 16], i16, tag="cols_sb")
        colp_sb = cpool.tile([128, NSETS], f32, tag="colp_sb")
        dre_sb = cpool.tile([128, NG], f32, tag="dre_sb")
        dissh = cpool.tile([128, SBLK], f32, tag="dissh")
        zkeep = cpool.tile([65, SBLK, 128], bf16, tag="zkeep")

        nc.sync.dma_start(out=iota[:], in_=iota_in[:])
        nc.sync.dma_start(out=identb[:], in_=identb_in[:])
        nc.sync.dma_start(out=sel2[:], in_=sel2_in[:])
        nc.sync.dma_start(out=sel3a[:], in_=sel3a_in[:])
        nc.sync.dma_start(out=sel3b[:], in_=sel3b_in[:])
        nc.sync.dma_start(out=W1t[:, 0, :], in_=W1_in[0:128, :])
        nc.sync.dma_start(out=W1t[:, 1, :], in_=W1_in[128:256, :])
        for l in (1, 2):
            nc.sync.dma_start(out=Wtb[l][:], in_=Wtb_in[l][:])
        for l in range(3):
            nc.sync.dma_start(out=Wvb[l][:], in_=Wvb_in[l][:])
        nc.sync.dma_start(out=W2b[:], in_=W2b_in[:])
        nc.sync.dma_start(out=rows_sb[:], in_=rows16[:])
        nc.sync.dma_start(out=cols_sb[:], in_=cols16[:])
        nc.sync.dma_start(out=colp_sb[:], in_=colp_in[:])
        nc.sync.dma_start(out=dre_sb[:], in_=dre_in[:])
        nc.sync.dma_start(out=dissh[:], in_=dissh_in[:])

        # ---------------- P1: h0 = relu(x @ W1 + b1) on own shard ----------
        preludeA = contextlib.ExitStack()
        psA = preludeA.enter_context(tc.tile_pool(name="psA", bufs=2, space="PSUM"))
        sbA = preludeA.enter_context(tc.tile_pool(name="sbA", bufs=3))

        onesr_sb = cpool.tile([1, 128], f32, tag="onesr")
        b1r_sb = cpool.tile([1, HID], f32, tag="b1r")
        nc.sync.dma_start(out=onesr_sb[:], in_=onesr_in[:])
        nc.sync.dma_start(out=b1r_sb[:], in_=b1r_in[:])
        b1p = psA.tile([128, HID], f32, tag="b1p")
        nc.tensor.matmul(out=b1p[:], lhsT=onesr_sb[:], rhs=b1r_sb[:],
                         start=True, stop=True)
        nc.vector.tensor_copy(out=b1tile[:], in_=b1p[:])

        hstg = None
        for t in range(SBLK):
            xf = sbA.tile([128, F_IN], f32, tag="xf")
            nc.sync.dma_start(out=xf[:], in_=x_sh[128 * t:128 * t + 128, :])
            xb = sbA.tile([128, F_IN], bf16, tag="xb")
            nc.vector.tensor_copy(out=xb[:], in_=xf[:])
            xT = sbA.tile([128, 2, 128], bf16, tag="xT")
            for k in range(2):
                tp = psA.tile([128, 128], bf16, tag="tp")
                nc.tensor.transpose(out=tp[:], in_=xb[:, 128 * k:128 * k + 128],
                                    identity=identb[:])
                nc.vector.tensor_copy(out=xT[:, k, :], in_=tp[:])
            hp = psA.tile([128, HID], f32, tag="hp")
            nc.tensor.matmul(out=hp[:], lhsT=xT[:, 0, :], rhs=W1t[:, 0, :],
                             start=True, stop=False)
            nc.tensor.matmul(out=hp[:], lhsT=xT[:, 1, :], rhs=W1t[:, 1, :],
                             start=False, stop=True)
            hs = sbA.tile([128, HID], f32, tag="hs")
            nc.vector.tensor_add(out=hs[:], in0=hp[:], in1=b1tile[:])
            if t % 4 == 0:
                hstg = sbA.tile([128, 4, HID], bf16, tag="hstg", name="hstg")
            nc.scalar.activation(out=hstg[:, t % 4, :], in_=hs[:], func=AF.Relu)
            if t % 4 == 3 or t == SBLK - 1:
                lo = t - t % 4
                ns = t % 4 + 1
                nc.sync.dma_start(
                    out=hsh0[128 * lo:128 * (t + 1), :].rearrange(
                        "(s p) w -> p s w", p=128),
                    in_=hstg[:, 0:ns, :])
        preludeA.close()
        nc.gpsimd.collective_compute(
            "AllGather", OP.bypass, replica_groups=RG,
            ins=[hsh0[:].opt()], outs=[hfull0[:].opt()])
        nc.sync.dma_start(out=H[:, 0:64],
                          in_=hfull0[:].rearrange("c s w -> (c s) w"))

        # ================= edge pass =================
        def edge_pass(l):
            L = l + 1
            nbat = EPAD // B
            estack = contextlib.ExitStack()
            gth = estack.enter_context(tc.tile_pool(name=f"gth{l}", bufs=3))
            cmp = estack.enter_context(tc.tile_pool(name=f"cmp{l}", bufs=2))
            aggs = estack.enter_context(
                tc.tile_pool(name=f"agg{l}", bufs=6, space="PSUM"))
            ohe = estack.enter_context(tc.tile_pool(name=f"ohe{l}", bufs=8))
            zst = estack.enter_context(tc.tile_pool(name=f"zst{l}", bufs=2))

            aggcur = {}
            zstg = None
            zw = ZW[l]
            Zt_v = Zt[l][:].rearrange("(c k p) w -> p c k w", c=NCORES,
                                      k=SBLK)
            for bi in range(nbat):
                e0 = bi * B
                bs = B
                gb = bs // 128
                idr = rows_sb[:, e0 // 16:(e0 + bs) // 16]
                idc = cols_sb[:, e0 // 16:(e0 + bs) // 16]
                hE = gth.tile([128, gb, EW[l]], bf16, tag="hE")
                nc.gpsimd.dma_gather(
                    out_ap=hE[:], in_ap=H[:, 0:EW[l]], idxs_ap=idr,
                    num_idxs=bs, num_idxs_reg=greg(bs), elem_size=EW[l],
                    elem_step=HW_TBL, single_packet=False)
                if l == 0:
                    nc.vector.memset(hE[:, :, 64:65], 1.0)
                if L > 1:
                    qE = gth.tile([128, gb, 128], bf16, tag="qE")
                    nc.gpsimd.dma_gather(
                        out_ap=qE[:], in_ap=qtab[l][:], idxs_ap=idc,
                        num_idxs=bs, num_idxs_reg=greg(bs), elem_size=128,
                        single_packet=False)
                    # prod[e, j, dh] = h_j[dh] * q~[dh]
                    prod = cmp.tile([128, gb, L, 64], bf16, tag="prod")
                    nc.vector.tensor_tensor(
                        out=prod[:],
                        in0=hE[:, :, 0:64 * L].rearrange(
                            "p g (j f) -> p g j f", j=L),
                        in1=qE[:, :, None, 0:64].to_broadcast(
                            [128, gb, L, 64]),
                        op=OP.mult)
                    # s[e, (g j), h] = sum_d prod
                    s = cmp.tile([128, gb * L, 8], f32, tag="s")
                    nc.vector.tensor_reduce(
                        out=s[:],
                        in_=prod[:].rearrange("p g j (d h) -> p (g j) h d",
                                              d=8),
                        axis=mybir.AxisListType.X, op=OP.add)
                    esc = cmp.tile([128, gb, L, 8], bf16, tag="esc")
                    nc.scalar.activation(
                        out=esc[:],
                        in_=s[:].rearrange("p (g j) h -> p g j h", j=L),
                        func=AF.Exp)
                    den = cmp.tile([128, gb, 8], f32, tag="den")
                    nc.vector.tensor_reduce(
                        out=den[:],
                        in_=esc[:].rearrange("p g j h -> p g h j"),
                        axis=mybir.AxisListType.X, op=OP.add)
                    rec = cmp.tile([128, gb, 8], bf16, tag="rec")
                    with nc.allow_low_precision(reason="softmax weights"):
                        nc.vector.reciprocal(out=rec[:], in_=den[:])
                    aw = cmp.tile([128, gb, L, 8], bf16, tag="aw")
                    nc.vector.tensor_tensor(
                        out=aw[:], in0=esc[:],
                        in1=rec[:, :, None, :].to_broadcast([128, gb, L, 8]),
                        op=OP.mult)
                    w = cmp.tile([128, gb, L, 64], bf16, tag="w")
                    for j in range(L):
                        nc.vector.tensor_tensor(
                            out=w[:, :, j, :].rearrange(
                                "p g (d h) -> p g d h", d=8),
                            in0=hE[:, :, 64 * j:64 * j + 64].rearrange(
                                "p g (d h) -> p g d h", d=8),
                            in1=aw[:, :, j, None, :].to_broadcast(
                                [128, gb, 8, 8]),
                            op=OP.mult)
                for c in range(gb):
                    g = e0 // 128 + c
                    for (p, fs, ls, si) in per_g[g]:
                        oh = ohe.tile([128, 128], bf16, tag="oh")
                        nc.vector.tensor_scalar(
                            out=oh[:], in0=iota[:],
                            scalar1=colp_sb[:, si:si + 1],
                            scalar2=dre_sb[:, g:g + 1],
                            op0=OP.is_equal, op1=OP.mult)
                        if fs:
                            aggcur[p] = aggs.tile([128, zw], f32, tag="aggp",
                                                  name="aggp")
                        if L == 1:
                            nc.tensor.matmul(out=aggcur[p][:], lhsT=oh[:],
                                             rhs=hE[:, c, 0:65],
                                             start=fs, stop=ls)
                        else:
                            for j in range(L):
                                nc.tensor.matmul(
                                    out=aggcur[p][:], lhsT=oh[:],
                                    rhs=w[:, c, j, :],
                                    start=(fs and j == 0),
                                    stop=(ls and j == L - 1))
                        if ls:
                            if p % 8 == 0:
                                zstg = zst.tile([128, 8, zw], bf16,
                                                tag="zstg", name="zstg")
                            nc.scalar.activation(out=zstg[:, p % 8, :],
                                                 in_=aggcur[p][:],
                                                 func=AF.Copy)
                            del aggcur[p]
                            if p % 8 == 7:
                                lb = p // 8
                                nc.sync.dma_start(out=Zt_v[:, :, lb, :],
                                                  in_=zstg[:])
            estack.close()
            nc.gpsimd.collective_compute(
                "ReduceScatter", OP.add, replica_groups=RG,
                ins=[Zt[l][:].opt()], outs=[Zrs[l][:].opt()])

        # ================= post-RS shard work =================
        def post(l):
            zw = ZW[l]
            pstack = contextlib.ExitStack()
            par = pstack.enter_context(tc.tile_pool(name=f"par{l}", bufs=3))
            pps = pstack.enter_context(
                tc.tile_pool(name=f"pps{l}", bufs=2, space="PSUM"))
            cstg = None
            lstg = None
            for lb in range(SBLK):
                zs = par.tile([128, zw], bf16, tag="zs")
                nc.sync.dma_start(out=zs[:],
                                  in_=Zrs[l][128 * lb:128 * lb + 128, :])
                ztp = pps.tile([zw, 128], bf16, tag="ztp")
                nc.tensor.transpose(out=ztp[:], in_=zs[:], identity=identb[:])
                nc.vector.tensor_copy(out=zkeep[0:zw, lb, :], in_=ztp[:])
                yp = pps.tile([128, HID], f32, tag="yp")
                nc.tensor.matmul(out=yp[:], lhsT=zkeep[:, lb, :],
                                 rhs=Wvb[l][:], start=True, stop=True)
                if l < 2:
                    if lb % 4 == 0:
                        cstg = par.tile([128, 4, 256], bf16, tag="cstg",
                                        name="cstg")
                    hrow = cstg[:, lb % 4, 0:64]
                    nc.scalar.activation(out=hrow, in_=yp[:], func=AF.Relu,
                                         scale=dissh[:, lb:lb + 1])
                    htp = pps.tile([64, 128], bf16, tag="htp")
                    nc.tensor.transpose(out=htp[:], in_=hrow,
                                        identity=identb[:])
                    hT65 = par.tile([65, 128], bf16, tag="hT65")
                    nc.vector.tensor_copy(out=hT65[0:64, :], in_=htp[:])
                    nc.vector.memset(hT65[64:65, :], 1.0)
                    qp = pps.tile([128, 64], f32, tag="qp")
                    nc.tensor.matmul(out=qp[:], lhsT=hT65[:],
                                     rhs=Wtb[l + 1][:], start=True, stop=True)
                    nc.scalar.activation(out=cstg[:, lb % 4, 64:128],
                                         in_=qp[:], func=AF.Copy)
                    nc.vector.tensor_copy(out=cstg[:, lb % 4, 128:192],
                                          in_=cstg[:, lb % 4, 64:128])
                    if lb % 4 == 3 or lb == SBLK - 1:
                        lo = lb - lb % 4
                        ns = lb % 4 + 1
                        nc.sync.dma_start(
                            out=csh[l + 1][128 * lo:128 * (lb + 1), :]
                            .rearrange("(s p) w -> p s w", p=128),
                            in_=cstg[:, 0:ns, :])
                else:
                    hrow3 = par.tile([128, HID], bf16, tag="hrow3")
                    nc.scalar.activation(out=hrow3[:], in_=yp[:], func=AF.Relu,
                                         scale=dissh[:, lb:lb + 1])
                    htp = pps.tile([64, 128], bf16, tag="htp")
                    nc.tensor.transpose(out=htp[:], in_=hrow3[:],
                                        identity=identb[:])
                    hT65 = par.tile([65, 128], bf16, tag="hT65")
                    nc.vector.tensor_copy(out=hT65[0:64, :], in_=htp[:])
                    nc.vector.memset(hT65[64:65, :], 1.0)
                    lgp = pps.tile([128, N_CLASS], f32, tag="lgp")
                    nc.tensor.matmul(out=lgp[:], lhsT=hT65[:], rhs=W2b[:],
                                     start=True, stop=True)
                    mx = par.tile([128, 1], f32, tag="mx")
                    nc.vector.tensor_reduce(out=mx[:], in_=lgp[:],
                                            axis=mybir.AxisListType.X,
                                            op=OP.max)
                    t1 = par.tile([128, N_CLASS], f32, tag="t1")
                    nc.vector.tensor_scalar(
                        out=t1[:], in0=lgp[:], scalar1=mx[:],
                        scalar2=None, op0=OP.subtract)
                    ex = par.tile([128, N_CLASS], f32, tag="ex")
                    sm = par.tile([128, 1], f32, tag="sm")
                    nc.scalar.activation(out=ex[:], in_=t1[:],
                                         func=AF.Exp, accum_out=sm[:])
                    lns = par.tile([128, 1], f32, tag="lns")
                    nc.scalar.activation(out=lns[:], in_=sm[:], func=AF.Ln)
                    if lb % 4 == 0:
                        lstg = par.tile([128, 4, N_CLASS], f32, tag="lstg",
                                        name="lstg")
                    nc.vector.tensor_scalar(
                        out=lstg[:, lb % 4, :], in0=t1[:], scalar1=lns[:],
                        scalar2=None, op0=OP.subtract)
                    if lb % 4 == 3 or lb == SBLK - 1:
                        lo = lb - lb % 4
                        ns = lb % 4 + 1
                        nc.sync.dma_start(
                            out=lg_out[128 * lo:128 * (lb + 1), :]
                            .rearrange("(s p) w -> p s w", p=128),
                            in_=lstg[:, 0:ns, :])
            pstack.close()
            if l < 2:
                nc.gpsimd.collective_compute(
                    "AllGather", OP.bypass, replica_groups=RG,
                    ins=[csh[l + 1][:].opt()], outs=[Comb[l + 1][:].opt()])
                nc.sync.dma_start(
                    out=H[:, 64 * (l + 1):64 * (l + 2)],
                    in_=Comb[l + 1][:].rearrange("c s w -> (c s) w")[:, 0:64])
                nc.sync.dma_start(
                    out=qtab[l + 1][:],
                    in_=Comb[l + 1][:].rearrange(
                        "c s w -> (c s) w")[:, 64:192])

        # ---------------- layers ----------------
        for l in range(N_LAYERS):
            edge_pass(l)
            post(l)

    # walrus: at most one sync-wait per instruction; split extras onto drains
    nsplit = 0
    for bb in nc.main_func.blocks:
        out = []
        for ins in list(bb.instructions):
            si = ins.sync_info
            if si is not None and si.on_wait and len(si.on_wait) > 1:
                waits = list(si.on_wait)
                k = 0
                while len(waits) > 1:
                    chunk, waits = waits[:1], waits[1:]
                    nop = mybir.InstDrain(
                        name=f"{ins.name}_ws{k}", engine=ins.engine,
                        ins=[], outs=[],
                        sync_info=mybir.SyncInfo(on_wait=chunk, on_update=[]))
                    nc.register_instruction(nop)
                    out.append(nop)
                    k += 1
                    nsplit += 1
                si.on_wait = waits
            out.append(ins)
        bb.instructions = out
    return nc


# ================================================================ entry
def _build_inmaps(inputs, ep):
    W = prep_weights(*[np.asarray(inputs[k]) for k in
                       ("lin1_w", "lin1_b", "wq", "bq", "wk", "bk",
                        "wv", "bv", "lin2_w", "lin2_b")])
    rows, cols, sets = ep["rows"], ep["cols"], ep["sets"]
    keys, NG = ep["keys"], ep["NG"]
    NSETS = len(sets)
    dis = host_norm(np.asarray(inputs["edge_index"]))

    xpad = np.zeros((NPAD, F_IN), np.float32)
    xpad[:N] = np.asarray(inputs["x"], np.float32)

    sel_a3, sel_b3 = _sel_matrices(3)
    sel_a2, _ = _sel_matrices(2)

    def stackb(w, b):
        return np.concatenate([w, b[None, :]], 0).astype(np_bf16)

    common = {
        "iota": np.tile(np.arange(128, dtype=np.float32)[None, :],
                        (128, 1)).astype(np_bf16),
        "identb": np.eye(128, dtype=np.float32).astype(np_bf16),
        "sel2": sel_a2.astype(np_bf16),
        "sel3a": sel_a3.astype(np_bf16),
        "sel3b": sel_b3.astype(np_bf16),
        "W1b": W["W1"].astype(np_bf16),
        "b1r": W["b1"][None, :].astype(np.float32),
        "onesr": np.ones((1, 128), np.float32),
        "W2b": stackb(W["W2"], W["b2"]),
    }
    for l in (1, 2):
        common[f"Wtb{l}"] = stackb(W[f"Wt{l}"], W[f"bt{l}"])
    for l in range(3):
        common[f"Wvb{l}"] = stackb(W[f"Wv{l}"], W[f"bv{l}"])

    in_maps = []
    for c in range(NCORES):
        colp = np.zeros((128, NSETS), np.float32)
        for si, (p, g, fs, ls) in enumerate(sets):
            kk = keys[c, 128 * g:128 * g + 128]
            colp[:, si] = np.where(kk >= 0, kk - 128 * p, -1)
        dre = dis[np.maximum(rows[c], 0)].reshape(NG, 128).T.astype(np.float32)
        nodes = SHARD * c + np.arange(SHARD)
        dissh = dis[nodes].reshape(SBLK, 128).T.astype(np.float32)
        m = dict(common)
        m["x_sh"] = xpad[SHARD * c:SHARD * (c + 1)].copy()
        m["rows16"] = _wrap16(np.maximum(rows[c], 0))
        m["cols16"] = _wrap16(np.maximum(cols[c], 0))
        m["colp"] = colp.astype(np.float32)
        m["dre"] = dre
        m["dissh"] = dissh
        in_maps.append(m)
    return in_maps


_CACHE = {}


def _patch_interp():
    from concourse import bass_interp
    if getattr(bass_interp, "_dna_isa_patch", False):
        return
    orig = bass_interp._visit_InstISA

    def patched(isa, instruction, core_sim):
        if int(instruction.isa_opcode) == 223:
            lib = (instruction.ant_dict or {}).get("lib_index", 0)
            core_sim.pool_library_index = lib
            return
        return orig(isa, instruction, core_sim)

    bass_interp._visit_InstISA = patched
    bass_interp._dna_isa_patch = True


def prepare(inputs):
    import sys
    if "/opt/trn_rl_repo" not in sys.path:
        sys.path.insert(0, "/opt/trn_rl_repo")
    _patch_interp()
    ep = prep_edges(np.asarray(inputs["edge_index"]))
    in_maps = _build_inmaps(inputs, ep)
    nc = build_nc(ep)
    return nc, in_maps


def kernel(**inputs):
    import sys
    if "/opt/trn_rl_repo" not in sys.path:
        sys.path.insert(0, "/opt/trn_rl_repo")
    from concourse.bass_utils import run_bass_kernel_spmd

    nc, in_maps = prepare(inputs)
    res = run_bass_kernel_spmd(nc, in_maps, core_ids=list(range(NCORES)))
    _CACHE["res"] = res
    logits = np.concatenate([res.results[c]["logits"]
                             for c in range(NCORES)], 0)
    return logits[:N].astype(np.float32)
